# revision 1
# baseline (speedup 1.0000x reference)
"""CondTransport kernel v2 for 8x Trainium2 NeuronCores.

Math (per reference):
  x_mean = [x_mu, y_mean+y_var]                      [Nq, 64]
  x_var  = [x_mu, 0.01*flip(y_eta), y_mean+y_var]    [Nq, 96]
  Lam_m  = kXXmean_inv @ Z_mean                      [Nx, 32]
  Lam_v  = kXXvar_inv  @ Z_var                       [Nx, 32]
  K_m    = exp(-d2(X_mean, x_mean)/128);  z_m = K_m.T @ Lam_m
  K_v    = exp(-d2(X_var,  x_var )/128);  z_v = K_v.T @ Lam_v
  out    = y_mean + y_var + z_m + z_v                [Nq, 32]

Sharding: queries across 8 cores (1024 each); Lambda rows across cores
(1024 each), split into G=4 groups of 256 rows that stream+gather
incrementally so z-accumulation overlaps the inv Gram DMA stream.

d2 trick: S~ = X.q - |X|^2/2 - |q|^2/2 computed in ONE matmul by
appending two contraction rows: X-side rows [.., -|X|^2/2, ones] and
q-side rows [.., ones, -|q|^2/2]. Then K = exp(S~/64) via ACT with
scale only; no bias APs, no post-scaling, and z_m + z_v accumulate
into one PSUM accumulator drained once.
"""
import sys

sys.path.insert(0, "/opt/trn_rl_repo")

import numpy as np
from contextlib import ExitStack

import concourse.bacc as bacc
import concourse.bass as bass
import concourse.masks as masks
import concourse.mybir as mybir
import concourse.tile as tile
from concourse.bass_utils import run_bass_kernel_spmd

NX = 8192
NQ = 8192
DX = 32
DY = 32
DM = 64          # x_mean feature dim
DV = 96          # x_var feature dim
DM2 = DM + 2     # + norm row + ones row
DV2 = DV + 2
NCORES = 8
QLOC = NQ // NCORES           # 1024 queries per core
RLOC = NX // NCORES           # 1024 Lambda rows per core
NXT = NX // 128               # 64 x-tiles
G = 4                         # lambda groups per matrix per core
GR = RLOC // G                # 256 lambda rows per group
GT = GR // 128                # 2 x-tiles per core contribution per group
NKC = 16                      # k-chunks per group (512 k each)
KSUB = 4                      # 128-k sub-tiles per chunk

F32 = mybir.dt.float32
F32R = mybir.dt.float32r
BF16 = mybir.dt.bfloat16
EXP = mybir.ActivationFunctionType.Exp
COPY = mybir.ActivationFunctionType.Copy

_CACHED_NC = None


def _build_nc():
    nc = bacc.Bacc("TRN2", target_bir_lowering=False, debug=False,
                   num_devices=NCORES)

    din = {}
    def inp(name, shape, dt=F32R):
        din[name] = nc.dram_tensor(name, list(shape), dt, kind="ExternalInput").ap()
        return din[name]

    # inv Gram slices, host pre-tiled to DMA-consumption order:
    # [G, NKC, 128, KSUB, GR] : chunk (g, kc) is contiguous 512KB
    invm = inp("invm", (G, NKC, 128, KSUB * GR))
    invv = inp("invv", (G, NKC, 128, KSUB * GR))
    XmT = inp("XmT", (DM, NX))            # X_mean.T (feature-major)
    XvT = inp("XvT", (DV, NX))            # X_var.T
    Zm = inp("Zm", (128, NXT * DY))       # host pre-tiled (t p) d -> p (t d)
    Zv = inp("Zv", (128, NXT * DY))
    xmuT = inp("xmuT", (DX, QLOC))        # local slice, transposed
    yefT = inp("yefT", (DY, QLOC))        # flip(y_eta).T slice (unscaled)
    ymT = inp("ymT", (DY, QLOC))
    yvT = inp("yvT", (DY, QLOC))
    ym_nat = inp("ym_nat", (QLOC, DY), F32)
    yv_nat = inp("yv_nat", (QLOC, DY), F32)

    out = nc.dram_tensor("out", [QLOC, DY], F32, kind="ExternalOutput").ap()

    warm_in = nc.dram_tensor("warm_in", [GR, DY], F32R, kind="Internal").ap()
    warm_out = nc.dram_tensor("warm_out", [NCORES * GR, DY], F32R,
                              kind="Internal", addr_space="Shared").ap()

    # collective bounce buffers per (matrix, group)
    lam_in = {}
    lam_out = {}
    for mat in "mv":
        for g in range(G):
            lam_in[mat, g] = nc.dram_tensor(
                f"lam_in_{mat}{g}", [GR, DY], F32R, kind="Internal").ap()
            lam_out[mat, g] = nc.dram_tensor(
                f"lam_out_{mat}{g}", [NCORES * GR, DY], F32R, kind="Internal",
                addr_space="Shared").ap()

    with tile.TileContext(nc) as tc, ExitStack() as ctx:
        P = lambda **kw: ctx.enter_context(tc.tile_pool(**kw))
        const_pool = P(name="const", bufs=1)
        inv_pool = P(name="inv", bufs=8)
        k_pool = P(name="ktile", bufs=6)
        work = P(name="work", bufs=2)
        psumS = P(name="psumS", bufs=2, space="PSUM")   # [128,1024] x2 = 4 banks
        psumZ = P(name="psumZ", bufs=1, space="PSUM")   # [32,1024] = 2 banks
        psumA = P(name="psumA", bufs=2, space="PSUM")   # [32,256]/[128,32] = 2 banks

        # ---------------- setup ----------------
        ident = const_pool.tile([128, 128], F32, tag="ident")
        masks.make_identity(nc, ident[:])

        Zm_sb = const_pool.tile([128, NXT * DY], F32R, tag="Zm_sb")
        nc.scalar.dma_start(Zm_sb[:], Zm)
        Zv_sb = const_pool.tile([128, NXT * DY], F32R, tag="Zv_sb")
        nc.scalar.dma_start(Zv_sb[:], Zv)

        XmT_sb = const_pool.tile([DM2, NX], F32R, tag="XmT_sb")
        nc.scalar.dma_start(XmT_sb[0:DM, :], XmT)
        XvT_sb = const_pool.tile([DV2, NX], F32R, tag="XvT_sb")
        nc.scalar.dma_start(XvT_sb[0:DV, :], XvT)

        # query feature slabs with the two extra rows
        qmT = const_pool.tile([DM2, QLOC], F32R, tag="qmT")
        nc.scalar.dma_start(qmT[0:DX, :], xmuT)
        nc.scalar.dma_start(qmT[DX:DM, :], ymT)
        yv_scr = const_pool.tile([DM, QLOC], F32R, tag="yv_scr")
        nc.scalar.dma_start(yv_scr[DX:DM, :], yvT)
        nc.vector.tensor_add(qmT[DX:DM, :], qmT[DX:DM, :], yv_scr[DX:DM, :])

        qvT = const_pool.tile([DV2, QLOC], F32R, tag="qvT")
        nc.scalar.dma_start(qvT[0:DX, :], xmuT)
        nc.scalar.dma_start(qvT[DX:DM, :], yefT)
        nc.vector.tensor_scalar_mul(qvT[DX:DM, :], qvT[DX:DM, :], 0.01)
        nc.vector.tensor_copy(qvT[DM:DV, :], qmT[DX:DM, :])  # y_mean+y_var

        neg_half_col = const_pool.tile([128, 1], F32R, tag="neg_half_col")
        nc.scalar.activation(neg_half_col[:], ident[:, 0:1], COPY,
                             bias=-0.5, scale=0.0)
        ones_row_sb = const_pool.tile([1, NX], F32R, tag="ones_row_sb")
        nc.scalar.activation(ones_row_sb[:], XmT_sb[0:1, :], COPY,
                             bias=1.0, scale=0.0)

        # Extra contraction rows. Engine writes need 32-aligned partition
        # bases, so row dfeat (64/96) is written directly and row dfeat+1
        # (65/97) goes through a partition-0 scratch + SBUF DMA.
        # X-side: row dfeat = -|X|^2/2, row dfeat+1 = ones.
        # q-side: row dfeat = ones,     row dfeat+1 = -|q|^2/2.
        def norm_chunk_x(T_sb, dfeat, cchunk):
            cs = slice(cchunk * 512, (cchunk + 1) * 512)
            sq = work.tile([dfeat, 512], F32R, tag="sq")
            nc.vector.tensor_mul(sq[:], T_sb[0:dfeat, cs], T_sb[0:dfeat, cs])
            pn = psumA.tile([1, 512], F32, tag="pa", name="pnorm")
            nc.tensor.matmul(pn[:], neg_half_col[0:dfeat, :], sq[:],
                             start=True, stop=True)
            nc.vector.tensor_copy(T_sb[dfeat:dfeat + 1, cs], pn[:])

        def norm_chunk_q(T_sb, dfeat, cchunk, key):
            cs = slice(cchunk * 512, (cchunk + 1) * 512)
            sq = work.tile([dfeat, 512], F32R, tag="sq")
            nc.vector.tensor_mul(sq[:], T_sb[0:dfeat, cs], T_sb[0:dfeat, cs])
            pn = psumA.tile([1, 512], F32, tag="pa", name="pnorm")
            nc.tensor.matmul(pn[:], neg_half_col[0:dfeat, :], sq[:],
                             start=True, stop=True)
            nc.vector.tensor_copy(qn_rows[key][:, cs], pn[:])

        # ones rows via DMA; norm chunks deferred into the first B slots
        nc.scalar.dma_start(XmT_sb[DM + 1:DM + 2, :], ones_row_sb[:])
        nc.scalar.dma_start(XvT_sb[DV + 1:DV + 2, :], ones_row_sb[:])
        nc.scalar.dma_start(qmT[DM:DM + 1, :], ones_row_sb[:, 0:QLOC])
        nc.scalar.dma_start(qvT[DV:DV + 1, :], ones_row_sb[:, 0:QLOC])
        qn_rows = {}
        for key, dfeat in (("qm", DM), ("qv", DV)):
            qn_rows[key] = const_pool.tile([1, QLOC], F32R, tag=f"qn_{key}",
                                           name=f"qn_{key}")

        norm_units = []
        for cchunk in range(NX // 512):
            norm_units.append(lambda c=cchunk: norm_chunk_x(XmT_sb, DM, c))
            norm_units.append(lambda c=cchunk: norm_chunk_x(XvT_sb, DV, c))
        for cchunk in range(QLOC // 512):
            norm_units.append(lambda c=cchunk: norm_chunk_q(qmT, DM, c, "qm"))
            norm_units.append(lambda c=cchunk: norm_chunk_q(qvT, DV, c, "qv"))

        def finish_norms():
            nc.scalar.dma_start(qmT[DM + 1:DM + 2, :], qn_rows["qm"][:])
            nc.scalar.dma_start(qvT[DV + 1:DV + 2, :], qn_rows["qv"][:])

        # lambda slabs: per (matrix, group): [128, 16 tiles * DY]
        lam_slab = {}
        for mat in "mv":
            for g in range(G):
                lam_slab[mat, g] = const_pool.tile(
                    [128, NCORES * GT * DY], BF16, tag=f"lam_{mat}{g}",
                    name=f"lam_slab_{mat}{g}")

        # z accumulator psum [32, 1024] over BOTH matrices
        pz = psumZ.tile([DY, QLOC], F32, tag="pz")

        # ---------------- pipelined stream ----------------
        # schedule: per (matrix, group): stream 16 inv chunks with 4
        # stage-A matmuls each, interleaved per-chunk with ONE phase-B
        # x-tile of the PREVIOUS group (16 chunks <-> 16 tiles). After a
        # group's stage A: transpose, ship, AllGather; phase B consumes
        # the gathered slab one group behind the stream.
        seq = [("m", g) for g in range(G)] + [("v", g) for g in range(G)]
        n_z_emitted = [0]

        # z emission lags one x-tile behind S/exp so the in-order PE never
        # waits on the ACT exp of the tile it just produced. start/stop are
        # tracked per psum bank region (per qc).
        pending_z = [None]   # (slab, slot, kt_tile)

        def emit_z():
            if pending_z[0] is None:
                return
            slab, slot, kt = pending_z[0]
            pending_z[0] = None
            nz = n_z_emitted[0]
            for qc in range(QLOC // 512):
                nc.tensor.matmul(
                    pz[:, qc * 512:(qc + 1) * 512],
                    slab[:, slot * DY:(slot + 1) * DY],
                    kt[:, qc * 512:(qc + 1) * 512],
                    start=(nz == 0),
                    stop=(nz == 2 * NXT - 1),
                    skip_group_check=True)
            n_z_emitted[0] += 1

        def emit_b_tile(mat, g, slot):
            # slot in [0, 16): core j = slot//GT contributes x-tile
            # T = 8*j + GT*g + slot%GT, at slab column slot*DY
            XT_sb = XmT_sb if mat == "m" else XvT_sb
            qT_sb = qmT if mat == "m" else qvT
            slab = lam_slab[mat, g]
            j_core, i = divmod(slot, GT)
            T = 8 * j_core + GT * g + i
            ps = psumS.tile([128, QLOC], F32, tag="ps")
            for qc in range(QLOC // 512):
                nc.tensor.matmul(
                    ps[:, qc * 512:(qc + 1) * 512],
                    XT_sb[:, T * 128:(T + 1) * 128],
                    qT_sb[:, qc * 512:(qc + 1) * 512],
                    start=True, stop=True)
            kt = k_pool.tile([128, QLOC], BF16, tag="ktile")
            nc.scalar.activation(kt[:], ps[:], EXP, scale=1.0 / 64.0)
            emit_z()
            pending_z[0] = (slab, slot, kt)

        def emit_group(cur, prev, widx):
            mat, g = cur
            Z_sb = Zm_sb if mat == "m" else Zv_sb
            inv_d = invm if mat == "m" else invv
            pa = psumA.tile([DY, GR], F32, tag="pa", name=f"pa_{mat}{g}")
            for kc in range(NKC):
                chunk = inv_pool.tile([128, KSUB * GR], F32R, tag="invchunk")
                nc.sync.dma_start(chunk[:], inv_d[g, kc])
                for s in range(KSUB):
                    kt_i = kc * KSUB + s
                    nc.tensor.matmul(
                        pa[:],
                        Z_sb[:, kt_i * DY:(kt_i + 1) * DY],
                        chunk[:, s * GR:(s + 1) * GR],
                        start=(kc == 0 and s == 0),
                        stop=(kc == NKC - 1 and s == KSUB - 1))
                if prev is not None:
                    emit_b_tile(prev[0], prev[1], kc)
                else:
                    step = widx * NKC + kc          # 0..31 over first 2 windows
                    lo = (step * len(norm_units)) // (2 * NKC)
                    hi = ((step + 1) * len(norm_units)) // (2 * NKC)
                    for u in norm_units[lo:hi]:
                        u()
                    if step == 2 * NKC - 1:
                        finish_norms()
            # transpose [32, 256] -> 2x [128, 32] natural, ship, gather
            lamT = work.tile([DY, GR], F32, tag="lamT")
            nc.vector.tensor_copy(lamT[:], pa[:])
            lam_nat = work.tile([128, GT * DY], F32R, tag="lam_nat")
            for j in range(GT):
                pt = psumA.tile([128, DY], F32, tag="pa", name=f"pt_{mat}{g}{j}")
                nc.tensor.transpose(pt[:], lamT[:, j * 128:(j + 1) * 128],
                                    ident[0:DY, 0:DY])
                nc.vector.tensor_copy(lam_nat[:, j * DY:(j + 1) * DY], pt[:])
            nc.scalar.dma_start(
                lam_in[mat, g].rearrange("(t p) d -> p t d", p=128), lam_nat[:])
            nc.gpsimd.collective_compute(
                "AllGather", mybir.AluOpType.bypass,
                replica_groups=[list(range(NCORES))],
                ins=[lam_in[mat, g].opt()], outs=[lam_out[mat, g].opt()])
            lam_stage = work.tile([128, NCORES * GT * DY], F32R,
                                  tag="lam_stage")
            nc.scalar.dma_start(
                lam_stage[:],
                lam_out[mat, g].rearrange("(t p) d -> p t d", p=128))
            nc.vector.tensor_copy(lam_slab[mat, g][:], lam_stage[:])

        for widx, cur in enumerate(seq):
            prev = seq[widx - 2] if widx >= 2 else None
            emit_group(cur, prev, widx)
        for tail in seq[-2:]:
            for slot in range(NCORES * GT):
                emit_b_tile(tail[0], tail[1], slot)
        emit_z()

        # ymv natural for the final combine: [128, 8*DY]
        ymv_sb = const_pool.tile([128, (QLOC // 128) * DY], F32, tag="ymv_sb")
        for j in range(QLOC // 128):
            t = work.tile([128, DY], F32, tag="ymv_t")
            nc.scalar.dma_start(t[:], ym_nat[j * 128:(j + 1) * 128, :])
            t2 = work.tile([128, DY], F32, tag="ymv_t2")
            nc.scalar.dma_start(t2[:], yv_nat[j * 128:(j + 1) * 128, :])
            nc.vector.tensor_add(ymv_sb[:, j * DY:(j + 1) * DY], t[:], t2[:])

        # ---------------- combine + output ----------------
        zT = const_pool.tile([DY, QLOC], F32, tag="zT")
        nc.vector.tensor_copy(zT[:], pz[:])
        out_sb = const_pool.tile([128, (QLOC // 128) * DY], F32, tag="out_sb")
        for j in range(QLOC // 128):
            pt = psumA.tile([128, DY], F32, tag="pa", name=f"ptz{j}")
            nc.tensor.transpose(pt[:], zT[:, j * 128:(j + 1) * 128],
                                ident[0:DY, 0:DY])
            sl = slice(j * DY, (j + 1) * DY)
            nc.vector.tensor_add(out_sb[:, sl], pt[:], ymv_sb[:, sl])
            nc.scalar.dma_start(out[j * 128:(j + 1) * 128, :], out_sb[:, sl])

    nc.compile()
    return nc


def get_nc():
    global _CACHED_NC
    if _CACHED_NC is None:
        _CACHED_NC = _build_nc()
    return _CACHED_NC


def _host_prep(x_mu, y_eta, y_mean, y_var, X_mean, X_var, Z_mean, Z_var,
               kXXmean_inv, kXXvar_inv):
    """Layout-only host prep: transposes / slicing / flip / inv pre-tiling."""
    C = np.ascontiguousarray
    XmT = C(X_mean.T)
    XvT = C(X_var.T)
    yef = y_eta[::-1]

    # pre-tile inv transposes into DMA-consumption order:
    # T[c][g, kc, p, s*GR + cw] = invT[kc*512 + s*128 + p, c*RLOC + g*GR + cw]
    def tile_inv(inv):
        invT = C(inv.T)                             # [k, r]
        V = invT.reshape(NKC, KSUB, 128, NCORES, G, GR)
        T = V.transpose(3, 4, 0, 2, 1, 5)           # [c, g, kc, p, s, cw]
        return C(T).reshape(NCORES, G, NKC, 128, KSUB * GR)

    invm_t = tile_inv(kXXmean_inv)
    invv_t = tile_inv(kXXvar_inv)

    def tile_z(Z):
        return C(Z.reshape(NXT, 128, DY).transpose(1, 0, 2).reshape(128, NXT * DY))

    Zm_t = tile_z(Z_mean)
    Zv_t = tile_z(Z_var)
    xmuT_f, yefT_f, ymT_f, yvT_f = C(x_mu.T), C(yef.T), C(y_mean.T), C(y_var.T)
    in_maps = []
    for c in range(NCORES):
        q = slice(c * QLOC, (c + 1) * QLOC)
        in_maps.append({
            "invm": invm_t[c],
            "invv": invv_t[c],
            "XmT": XmT, "XvT": XvT,
            "Zm": Zm_t, "Zv": Zv_t,
            "xmuT": C(xmuT_f[:, q]), "yefT": C(yefT_f[:, q]),
            "ymT": C(ymT_f[:, q]), "yvT": C(yvT_f[:, q]),
            "ym_nat": C(y_mean[q]), "yv_nat": C(y_var[q]),
        })
    return in_maps


def kernel(x_mu, y_eta, y_mean, y_var, X_mean, X_var, Z_mean, Z_var,
           kXXmean_inv, kXXvar_inv, _trace=False, _tmpdir=None):
    nc = get_nc()
    in_maps = _host_prep(x_mu, y_eta, y_mean, y_var, X_mean, X_var,
                         Z_mean, Z_var, kXXmean_inv, kXXvar_inv)
    res = run_bass_kernel_spmd(nc, in_maps, core_ids=list(range(NCORES)),
                               trace=_trace, tmpdir=_tmpdir)
    out = np.concatenate([res.results[c]["out"] for c in range(NCORES)], axis=0)
    if _trace:
        kernel._last_results = res
    return out



# revision 9
# speedup vs baseline: 1.3785x; 1.3785x over previous
"""CondTransport kernel v3 for 8x Trainium2 NeuronCores.

Math (per reference):
  x_mean = [x_mu, y_mean+y_var]                      [Nq, 64]
  x_var  = [x_mu, 0.01*flip(y_eta), y_mean+y_var]    [Nq, 96]
  Lam_m  = kXXmean_inv @ Z_mean                      [Nx, 32]
  Lam_v  = kXXvar_inv  @ Z_var                       [Nx, 32]
  K_m    = exp(-d2(X_mean, x_mean)/128);  z_m = K_m.T @ Lam_m
  K_v    = exp(-d2(X_var,  x_var )/128);  z_v = K_v.T @ Lam_v
  out    = y_mean + y_var + z_m + z_v                [Nq, 32]

v3 changes vs v2 (418us baseline):
  * all matmul operands bf16 (host-cast): inv-Gram DMA halves to 32MB/core
    and S-matmuls stream at 1 col/cycle (f32r measured ~1.55x slower).
  * ScalarE queue carries ONLY the 128 exp ACTs -- the serial floor
    (~147us) -- all DMAs move to sync (inv stream) / gpsimd (rest).
  * producer/consumer decoupling: S+exp tiles are produced at a steady
    1/slot from early window 0 (kt ring 32 deep); z matmuls consume
    kt >=6 tiles behind and >=2 windows behind the Lambda gather, so the
    in-order PE never blocks on the ACT or the collective.
  * Lambda and z matmuls (M=32) are 4-way column-tiled across PE column
    groups (out base partition 32j auto-derives tile_position), cutting
    their PE cost ~3x.
  * d2 trick unchanged: S~ = X.q - |X|^2/2 - |q|^2/2 via two extra
    contraction rows; X/q norm rows computed on-device (DVE square +
    PE reduction against -0.5 column), with the tiny [1,512] psum
    partials aliased into unused pz4 space during windows 0-1.
"""
import sys

sys.path.insert(0, "/opt/trn_rl_repo")

import numpy as np
import ml_dtypes
from contextlib import ExitStack

import concourse.bacc as bacc
import concourse.mybir as mybir
import concourse.tile as tile
from concourse.bass_utils import run_bass_kernel_spmd

NX = 8192
NQ = 8192
DX = 32
DY = 32
DM = 64          # x_mean feature dim
DV = 96          # x_var feature dim
DM2 = DM + 2     # + norm row + ones row
DV2 = DV + 2
NCORES = 8
QLOC = NQ // NCORES           # 1024 queries per core
RLOC = NX // NCORES           # 1024 Lambda rows per core
NXT = NX // 128               # 64 x-tiles
G = 4                         # lambda groups per matrix per core
GR = RLOC // G                # 256 lambda rows per group
GT = GR // 128                # 2 x-tiles per core contribution per group
NKC = 16                      # k-chunks per group (512 k each)
KSUB = 4                      # 128-k sub-tiles per chunk
NTILE = 2 * NXT               # 128 B-tiles total (m then v)

F32 = mybir.dt.float32
BF16 = mybir.dt.bfloat16
EXP = mybir.ActivationFunctionType.Exp

_CACHED_NC = None


def _build_nc():
    nc = bacc.Bacc("TRN2", target_bir_lowering=False, debug=False,
                   num_devices=NCORES)

    din = {}
    def inp(name, shape, dt=BF16):
        din[name] = nc.dram_tensor(name, list(shape), dt, kind="ExternalInput").ap()
        return din[name]

    # inv Gram slices, host pre-tiled to DMA-consumption order:
    # [G, NKC, 128, KSUB, GR] : chunk (g, kc) is contiguous 256KB bf16
    invm = inp("invm", (G, NKC, 128, KSUB * GR))
    invv = inp("invv", (G, NKC, 128, KSUB * GR))
    XmT = inp("XmT", (DM2, NX))           # X_mean.T + zero row + ones row
    XvT = inp("XvT", (DV2, NX))           # X_var.T + zero row + ones row
    Zm = inp("Zm", (128, NXT * DY))       # host pre-tiled (t p) d -> p (t d)
    Zv = inp("Zv", (128, NXT * DY))
    xmuT = inp("xmuT", (DX, QLOC))        # local slice, transposed
    yefT = inp("yefT", (DY, QLOC))        # flip(y_eta).T slice (unscaled)
    ymT = inp("ymT", (DY, QLOC))
    yvT = inp("yvT", (DY, QLOC))
    ones_q = inp("ones_q", (1, QLOC))
    neg_half = inp("neg_half", (128, 1))
    ident = inp("ident", (128, 128), F32)
    ym_nat = inp("ym_nat", (QLOC, DY), F32)
    yv_nat = inp("yv_nat", (QLOC, DY), F32)

    out = nc.dram_tensor("out", [QLOC, DY], F32, kind="ExternalOutput").ap()

    # collective bounce buffers per (matrix, group)
    lam_in = {}
    lam_out = {}
    for mat in "mv":
        for g in range(G):
            lam_in[mat, g] = nc.dram_tensor(
                f"lam_in_{mat}{g}", [GR, DY], F32, kind="Internal").ap()
            lam_out[mat, g] = nc.dram_tensor(
                f"lam_out_{mat}{g}", [NCORES * GR, DY], F32, kind="Internal",
                addr_space="Shared").ap()

    with tile.TileContext(nc) as tc, ExitStack() as ctx:
        P = lambda **kw: ctx.enter_context(tc.tile_pool(**kw))
        const_pool = P(name="const", bufs=1)
        inv_pool = P(name="inv", bufs=8)
        k_pool = P(name="ktile", bufs=32)
        work = P(name="work", bufs=2)
        psumS = P(name="psumS", bufs=2, space="PSUM")   # [128,1024] x2 = 4 banks
        psumZ = P(name="psumZ", bufs=1, space="PSUM")   # [128,1024] = 2 banks
        psumA = P(name="psumA", bufs=2, space="PSUM")   # [128,256]/[128,32] x2

        # ---------------- constants / inputs ----------------
        ident_sb = const_pool.tile([128, 128], F32, tag="ident_sb")
        nc.gpsimd.dma_start(ident_sb[:], ident)
        nh_sb = const_pool.tile([128, 1], BF16, tag="nh_sb")
        nc.gpsimd.dma_start(nh_sb[:], neg_half)

        # dummy exp to pull the ACT table load into the startup window
        warm_sc = const_pool.tile([1, 1], F32, tag="warm_sc")
        nc.scalar.activation(warm_sc[:], nh_sb[0:1, :], EXP)

        Zm_sb = const_pool.tile([128, NXT * DY], BF16, tag="Zm_sb")
        nc.gpsimd.dma_start(Zm_sb[:], Zm)
        Zv_sb = const_pool.tile([128, NXT * DY], BF16, tag="Zv_sb")
        nc.gpsimd.dma_start(Zv_sb[:], Zv)

        XmT_sb = const_pool.tile([DM2, NX], BF16, tag="XmT_sb")
        nc.gpsimd.dma_start(XmT_sb[:], XmT)
        XvT_sb = const_pool.tile([DV2, NX], BF16, tag="XvT_sb")
        nc.gpsimd.dma_start(XvT_sb[:], XvT)

        # query feature slabs (norm row dfeat, ones row dfeat+1 on X side;
        # ones row dfeat, norm row dfeat+1 on q side)
        qmT = const_pool.tile([DM2, QLOC], BF16, tag="qmT")
        nc.gpsimd.dma_start(qmT[0:DX, :], xmuT)
        nc.gpsimd.dma_start(qmT[DX:DM, :], ymT)
        nc.gpsimd.dma_start(qmT[DM:DM + 1, :], ones_q)
        yv_tmp = const_pool.tile([DM, QLOC], BF16, tag="yv_tmp")
        nc.gpsimd.dma_start(yv_tmp[DX:DM, :], yvT)
        nc.vector.tensor_add(qmT[DX:DM, :], qmT[DX:DM, :], yv_tmp[DX:DM, :])

        qvT = const_pool.tile([DV2, QLOC], BF16, tag="qvT")
        nc.gpsimd.dma_start(qvT[0:DX, :], xmuT)
        nc.gpsimd.dma_start(qvT[DX:DM, :], yefT)
        nc.gpsimd.dma_start(qvT[DV:DV + 1, :], ones_q)
        nc.vector.tensor_scalar_mul(qvT[DX:DM, :], qvT[DX:DM, :], 0.01)
        nc.vector.tensor_copy(qvT[DM:DV, :], qmT[DX:DM, :])  # y_mean+y_var

        # lambda slabs: per (matrix, group): [128, 16 slots * DY]
        lam_slab = {}
        for mat in "mv":
            for g in range(G):
                lam_slab[mat, g] = const_pool.tile(
                    [128, NCORES * GT * DY], BF16, tag=f"lam_{mat}{g}",
                    name=f"lam_slab_{mat}{g}")

        # z accumulator psum [128, 1024]: 4 column-group bands of 32
        pz4 = psumZ.tile([128, QLOC], F32, tag="pz4")

        qn_rows = {}
        for key in ("qm", "qv"):
            qn_rows[key] = const_pool.tile([1, QLOC], BF16, tag=f"qn_{key}",
                                           name=f"qn_{key}")

        # ---------------- norm units ----------------
        # pn partials alias into pz4 (unused until window 2); rotate 4
        # disjoint [1,512] regions to keep the chain off the critical path.
        pn_regions = [(0, 0), (0, 512)]
        pn_ctr = [0]

        def pn_slot():
            p0, c0 = pn_regions[pn_ctr[0] % 2]
            pn_ctr[0] += 1
            return pz4[p0:p0 + 1, c0:c0 + 512]

        def norm_chunk_x(T_sb, dfeat, cchunk):
            cs = slice(cchunk * 512, (cchunk + 1) * 512)
            sq = work.tile([DV, 512], BF16, tag="sq", name="sq_x")
            nc.vector.tensor_mul(sq[0:dfeat, :], T_sb[0:dfeat, cs],
                                 T_sb[0:dfeat, cs])
            pn = pn_slot()
            nc.tensor.matmul(pn, nh_sb[0:dfeat, :], sq[0:dfeat, :],
                             start=True, stop=True, skip_group_check=True)
            nc.vector.tensor_copy(T_sb[dfeat:dfeat + 1, cs], pn)

        def norm_chunk_q(T_sb, dfeat, cchunk, key):
            cs = slice(cchunk * 512, (cchunk + 1) * 512)
            sq = work.tile([DV, 512], BF16, tag="sq", name="sq_q")
            nc.vector.tensor_mul(sq[0:dfeat, :], T_sb[0:dfeat, cs],
                                 T_sb[0:dfeat, cs])
            pn = pn_slot()
            nc.tensor.matmul(pn, nh_sb[0:dfeat, :], sq[0:dfeat, :],
                             start=True, stop=True, skip_group_check=True)
            nc.vector.tensor_copy(qn_rows[key][:, cs], pn)

        def finish_qnorms():
            nc.gpsimd.dma_start(qmT[DM + 1:DM + 2, :], qn_rows["qm"][:])
            nc.gpsimd.dma_start(qvT[DV + 1:DV + 2, :], qn_rows["qv"][:])

        # schedule: q units at slots 0-1; X units 2/slot from slot 2 in
        # producer-need order (m evens, v evens, m odds, v odds)
        norm_sched = {0: [], 1: []}
        for cc in range(2):
            norm_sched[0].append(lambda c=cc: norm_chunk_q(qmT, DM, c, "qm"))
            norm_sched[1].append(lambda c=cc: norm_chunk_q(qvT, DV, c, "qv"))
        norm_sched[1].append(finish_qnorms)
        xunits = []
        for par in (0, 1):
            for mat in "mv":
                T_sb, dfeat = (XmT_sb, DM) if mat == "m" else (XvT_sb, DV)
                for j in range(8):
                    cchunk = 2 * j + par
                    xunits.append(lambda t=T_sb, d=dfeat, c=cchunk:
                                  norm_chunk_x(t, d, c))
        for i, u in enumerate(xunits):
            norm_sched.setdefault(2 + i // 2, []).append(u)

        # ---------------- producer: S matmuls + exp ----------------
        seq = [("m", g) for g in range(G)] + [("v", g) for g in range(G)]
        pending = []          # (pidx, mat, g, slot, kt)

        def emit_s_tile(pidx):
            mat, g = seq[pidx // 16]
            slot = pidx % 16
            T = 8 * (slot // GT) + GT * g + slot % GT
            XT_sb, dfeat = (XmT_sb, DM2) if mat == "m" else (XvT_sb, DV2)
            qT_sb = qmT if mat == "m" else qvT
            ps = psumS.tile([128, QLOC], F32, tag="ps")
            for qc in range(2):
                cs = slice(qc * 512, (qc + 1) * 512)
                nc.tensor.matmul(ps[:, cs], XT_sb[:, T * 128:(T + 1) * 128],
                                 qT_sb[:, cs], start=True, stop=True)
            kt = k_pool.tile([128, QLOC], BF16, tag="ktile")
            nc.scalar.activation(kt[:], ps[:], EXP, scale=1.0 / 64.0)
            pending.append((pidx, mat, g, slot, kt))

        # ---------------- consumer: z matmul blocks ----------------
        # band = pidx % 4 -> pz4[32b:32b+32]; 32 accumulating MMs per
        # (band, qc) across both matrices.
        zcnt = {}

        def emit_z_block():
            blk = pending[:4]
            del pending[:4]
            for qc in range(2):
                cs = slice(qc * 512, (qc + 1) * 512)
                for pidx, mat, g, slot, kt in blk:
                    b = pidx % 4
                    n = zcnt.get((b, qc), 0)
                    zcnt[(b, qc)] = n + 1
                    nc.tensor.matmul(
                        pz4[32 * b:32 * (b + 1), cs],
                        lam_slab[mat, g][:, slot * DY:(slot + 1) * DY],
                        kt[:, cs],
                        start=(n == 0), stop=(n == 31),
                        skip_group_check=True, tile_position=(0, 32 * b))

        # ---------------- lambda stream ----------------
        def emit_lambda_chunk(mat, g, kc, pa4):
            Z_sb = Zm_sb if mat == "m" else Zv_sb
            inv_d = invm if mat == "m" else invv
            chunk = inv_pool.tile([128, KSUB * GR], BF16, tag="invchunk")
            nc.sync.dma_start(chunk[:], inv_d[g, kc])
            for s in range(KSUB):
                kt_i = kc * KSUB + s
                nc.tensor.matmul(
                    pa4[32 * s:32 * (s + 1), :],
                    Z_sb[:, kt_i * DY:(kt_i + 1) * DY],
                    chunk[:, s * GR:(s + 1) * GR],
                    start=(kc == 0), stop=(kc == NKC - 1),
                    skip_group_check=True, tile_position=(0, 32 * s))

        def finish_window(mat, g, pa4):
            # move accumulator to SBUF (frees psum slot), transpose each
            # 128-col block so the 4 bands land on the free axis, then
            # partition-matched adds produce lam in natural layout
            sb4 = work.tile([128, GR], F32, tag="sb4")
            nc.vector.tensor_copy(sb4[:], pa4[:])
            lam_nat = work.tile([128, GT * DY], F32, tag="lam_nat")
            for j in range(GT):
                tp = psumA.tile([128, 128], F32, tag="pa", name=f"tp_{mat}{g}{j}")
                nc.tensor.transpose(tp[:], sb4[:, j * 128:(j + 1) * 128],
                                    ident_sb[:])
                acc = work.tile([128, DY], F32, tag="lacc")
                nc.vector.tensor_copy(acc[:], tp[:, 0:DY])
                nc.vector.tensor_add(acc[:], acc[:], tp[:, DY:2 * DY])
                nc.vector.tensor_add(acc[:], acc[:], tp[:, 2 * DY:3 * DY])
                nc.vector.tensor_add(lam_nat[:, j * DY:(j + 1) * DY],
                                     acc[:], tp[:, 3 * DY:4 * DY])
            nc.gpsimd.dma_start(
                lam_in[mat, g].rearrange("(t p) d -> p t d", p=128), lam_nat[:])
            nc.gpsimd.collective_compute(
                "AllGather", mybir.AluOpType.bypass,
                replica_groups=[list(range(NCORES))],
                ins=[lam_in[mat, g].opt()], outs=[lam_out[mat, g].opt()])
            lam_stage = work.tile([128, NCORES * GT * DY], F32,
                                  tag="lam_stage")
            nc.gpsimd.dma_start(
                lam_stage[:],
                lam_out[mat, g].rearrange("(t p) d -> p t d", p=128))
            nc.vector.tensor_copy(lam_slab[mat, g][:], lam_stage[:])

        # ---------------- main loop ----------------
        ymv_sb = const_pool.tile([128, (QLOC // 128) * DY], F32, tag="ymv_sb")

        def load_ymv():
            for j in range(QLOC // 128):
                t = work.tile([128, DY], F32, tag="ymv_t")
                nc.gpsimd.dma_start(t[:], ym_nat[j * 128:(j + 1) * 128, :])
                t2 = work.tile([128, DY], F32, tag="ymv_t2")
                nc.gpsimd.dma_start(t2[:], yv_nat[j * 128:(j + 1) * 128, :])
                nc.vector.tensor_add(ymv_sb[:, j * DY:(j + 1) * DY],
                                     t[:], t2[:])

        producer_idx = 0
        for wi, (mat, g) in enumerate(seq):
            pa4 = psumA.tile([128, GR], F32, tag="pa", name=f"pa4_{mat}{g}")
            for kc in range(NKC):
                emit_lambda_chunk(mat, g, kc, pa4)
                gslot = wi * NKC + kc
                for u in norm_sched.get(gslot, []):
                    u()
                if gslot >= 4 and producer_idx < NTILE:
                    emit_s_tile(producer_idx)
                    producer_idx += 1
                blocks = 0
                while (blocks < 2 and len(pending) >= 6
                       and pending[0][0] // 16 <= wi - 2):
                    emit_z_block()
                    blocks += 1
            finish_window(mat, g, pa4)
            if wi == 5:
                load_ymv()

        # drain producer + consumer
        while producer_idx < NTILE:
            emit_s_tile(producer_idx)
            producer_idx += 1
        while pending:
            emit_z_block()

        # ---------------- combine + output ----------------
        # same transposed-band reduction: per 128-query block, transpose
        # pz4 columns so bands land on free axis, sum, add ymv, store
        zb4 = const_pool.tile([128, QLOC], F32, tag="zb4")
        nc.vector.tensor_copy(zb4[:], pz4[:])
        out_sb = const_pool.tile([128, (QLOC // 128) * DY], F32, tag="out_sb")
        for j in range(QLOC // 128):
            tp = psumA.tile([128, 128], F32, tag="pa", name=f"tpz{j}")
            nc.tensor.transpose(tp[:], zb4[:, j * 128:(j + 1) * 128],
                                ident_sb[:])
            acc = work.tile([128, DY], F32, tag="zacc")
            nc.vector.tensor_copy(acc[:], tp[:, 0:DY])
            nc.vector.tensor_add(acc[:], acc[:], tp[:, DY:2 * DY])
            nc.vector.tensor_add(acc[:], acc[:], tp[:, 2 * DY:3 * DY])
            nc.vector.tensor_add(acc[:], acc[:], tp[:, 3 * DY:4 * DY])
            sl = slice(j * DY, (j + 1) * DY)
            nc.vector.tensor_add(out_sb[:, sl], acc[:], ymv_sb[:, sl])
            nc.gpsimd.dma_start(out[j * 128:(j + 1) * 128, :], out_sb[:, sl])

    nc.compile()
    return nc


def get_nc():
    global _CACHED_NC
    if _CACHED_NC is None:
        _CACHED_NC = _build_nc()
    return _CACHED_NC


def _host_prep(x_mu, y_eta, y_mean, y_var, X_mean, X_var, Z_mean, Z_var,
               kXXmean_inv, kXXvar_inv):
    """Layout-only host prep: transposes / slicing / flip / tiling, plus
    lossless-layout bf16 casts of the matmul operands."""
    BF = ml_dtypes.bfloat16
    C = np.ascontiguousarray

    def xslab(X, dfeat):
        # [dfeat+2, NX]: features.T, zero row (device-computed norm), ones
        s = np.zeros((dfeat + 2, NX), dtype=BF)
        s[0:dfeat] = X.T.astype(BF)
        s[dfeat + 1] = np.ones(NX, dtype=BF)
        return s

    XmT = xslab(X_mean, DM)
    XvT = xslab(X_var, DV)
    yef = y_eta[::-1]

    # pre-tile inv transposes into DMA-consumption order:
    # T[c][g, kc, p, s*GR + cw] = invT[kc*512 + s*128 + p, c*RLOC + g*GR + cw]
    def tile_inv(inv):
        invT = C(inv.astype(BF).T)                  # [k, r]
        V = invT.reshape(NKC, KSUB, 128, NCORES, G, GR)
        T = V.transpose(3, 4, 0, 2, 1, 5)           # [c, g, kc, p, s, cw]
        return C(T).reshape(NCORES, G, NKC, 128, KSUB * GR)

    invm_t = tile_inv(kXXmean_inv)
    invv_t = tile_inv(kXXvar_inv)

    def tile_z(Z):
        return C(Z.astype(BF).reshape(NXT, 128, DY).transpose(1, 0, 2)
                 .reshape(128, NXT * DY))

    Zm_t = tile_z(Z_mean)
    Zv_t = tile_z(Z_var)
    xmuT_f = C(x_mu.T.astype(BF))
    yefT_f = C(yef.T.astype(BF))
    ymT_f = C(y_mean.T.astype(BF))
    yvT_f = C(y_var.T.astype(BF))
    ones_q = np.ones((1, QLOC), dtype=BF)
    neg_half = np.full((128, 1), -0.5, dtype=BF)
    ident = np.eye(128, dtype=np.float32)
    in_maps = []
    for c in range(NCORES):
        q = slice(c * QLOC, (c + 1) * QLOC)
        in_maps.append({
            "invm": invm_t[c],
            "invv": invv_t[c],
            "XmT": XmT, "XvT": XvT,
            "Zm": Zm_t, "Zv": Zv_t,
            "xmuT": C(xmuT_f[:, q]), "yefT": C(yefT_f[:, q]),
            "ymT": C(ymT_f[:, q]), "yvT": C(yvT_f[:, q]),
            "ones_q": ones_q, "neg_half": neg_half, "ident": ident,
            "ym_nat": C(y_mean[q]), "yv_nat": C(y_var[q]),
        })
    return in_maps


def kernel(x_mu, y_eta, y_mean, y_var, X_mean, X_var, Z_mean, Z_var,
           kXXmean_inv, kXXvar_inv, _trace=False, _tmpdir=None):
    nc = get_nc()
    in_maps = _host_prep(x_mu, y_eta, y_mean, y_var, X_mean, X_var,
                         Z_mean, Z_var, kXXmean_inv, kXXvar_inv)
    res = run_bass_kernel_spmd(nc, in_maps, core_ids=list(range(NCORES)),
                               trace=_trace, tmpdir=_tmpdir)
    out = np.concatenate([res.results[c]["out"] for c in range(NCORES)], axis=0)
    if _trace:
        kernel._last_results = res
    return out
